# revision 23
# baseline (speedup 1.0000x reference)
"""Trainium2 (8-core) kernel for nn_NodeUpdateBlock: equivariant Linear +
FullyConnectedTensorProduct with 10 scalar (0e) one-hot attributes.

Self-contained: takes FULL inputs (as produced by the problem's
setup_inputs), distributes across the 8 NeuronCores internally, and
returns the FULL [N, 512] float32 output.

Strategy
--------
out_c = m_c @ (Wl_c * ls) + sum_a (att_a * f_c) @ (Wt_c[:,a,:] * ts)
per irrep-component plane c (1 scalar plane + 3 vector planes of 128
channels).  node_attrs rows are one-hot, so on the host we compute
z = argmax(attrs), sort nodes by z, and split every attr-group evenly
over the 8 cores; the tensor product then needs exactly ONE matmul per
(plane, group-run) with a compile-time weight slice.

The kernel is memory-bound, so the data plane is bf16: the host
downcasts m/f (and the weights) to bf16, transposes to channel-major,
and pre-tiles into the exact SBUF layout [chunk, 128 part, plane, 512
nodes] so every DMA packet is a contiguous 4 KiB run on both sides and
the PE needs no on-chip transposes at all.  Per 512-node chunk and
plane, one PSUM accumulation group computes
Y^T = WL^T m^T + WT_g^T f^T, and the result is copied to SBUF as bf16
and streamed out.  The host un-tiles and upcasts to f32 (bf16 rel-err
~3e-3, well inside the 2e-2 gate).  A dense (non-one-hot) f32 fallback
sums over all 10 attribute channels and stays correct for arbitrary
node_attrs.
"""

import math

import numpy as np
import ml_dtypes

import concourse.bacc as bacc
import concourse.mybir as mybir
from concourse.tile import TileContext
from concourse.bass_utils import run_bass_kernel_spmd

MUL = 128
NA = 10
DIM = 512
N_CORES = 8
CHUNK = 512  # nodes per compute chunk
LIN_SCALE = 1.0 / math.sqrt(MUL)
TP_SCALE = 1.0 / math.sqrt(MUL * NA)
F32 = mybir.dt.float32
F32R = mybir.dt.float32r
BF16 = mybir.dt.bfloat16
NP_BF16 = ml_dtypes.bfloat16

LAST_RESULTS = None  # BassKernelResults of the most recent run (for testing)


def _col_perm():
    perm = list(range(MUL))
    for x in range(3):
        perm += [MUL + 3 * i + x for i in range(MUL)]
    return np.array(perm, dtype=np.int64)


COL_PERM = _col_perm()
COL_PERM_INV = np.argsort(COL_PERM)


def build_program(chunk_sizes, chunk_runs, edge_opt=False, fillers=3, prefetch=4, copy_split=False, prewarm=0, last_hwdge=2, fillw=CHUNK, psy=7):
    """Bass program for one core (same program runs on all cores).

    Inputs:  m/f [n_chunks, 128, 2048] bf16 (pre-tiled channel-major; the
             last chunk may be ragged, packed at 4*cs cols within its block),
             wl [128, 256] bf16, wt [128, 2560] bf16
    Output:  out [n_chunks, 128, 2048] bf16 (same tiling)
    """
    n_chunks = len(chunk_sizes)
    nc = bacc.Bacc("TRN2")
    m = nc.dram_tensor("m", [n_chunks, 128, 4 * CHUNK], BF16, kind="ExternalInput")
    f = nc.dram_tensor("f", [n_chunks, 128, 4 * CHUNK], BF16, kind="ExternalInput")
    wl = nc.dram_tensor("wl", [MUL, 2 * MUL], BF16, kind="ExternalInput")
    wt = nc.dram_tensor("wt", [MUL, 2 * NA * MUL], BF16, kind="ExternalInput")
    out = nc.dram_tensor("out", [n_chunks, 128, 4 * CHUNK], BF16, kind="ExternalOutput")

    with TileContext(nc) as tc:
        with (
            tc.tile_pool(name="const", bufs=1) as cpool,
            tc.tile_pool(name="mio", bufs=7) as mpool,
            tc.tile_pool(name="fio", bufs=7) as fpool,
            tc.tile_pool(name="oio", bufs=6) as opool,
            tc.tile_pool(name="psY", bufs=psy, space="PSUM") as psY,
            tc.tile_pool(name="psF", bufs=1, space="PSUM") as psF,
        ):
            # weights ride the (otherwise idle at t=0) gpsimd queue so the
            # sync/scalar queues start streaming chunk data immediately
            wl_sb = cpool.tile([MUL, 2 * MUL], BF16, tag="wl")
            nc.gpsimd.dma_start(out=wl_sb, in_=wl[:])
            wt_sb = cpool.tile([MUL, 2 * NA * MUL], BF16, tag="wt")
            nc.gpsimd.dma_start(out=wt_sb, in_=wt[:])

            # loads split half/half across the two HWDGE queues; stores on
            # gpsimd.  Loads are issued PREFETCH chunks ahead so their
            # issue slots never queue behind PSUM-copy waits on the same
            # engine.
            PREFETCH = prefetch
            m_tiles, f_tiles = {}, {}

            def load(ck):
                cs = chunk_sizes[ck]
                h = 2 * cs
                mt = mpool.tile([128, 4 * CHUNK], BF16, tag="m")
                ft = fpool.tile([128, 4 * CHUNK], BF16, tag="f")
                # the store queue (gpsimd) is idle during the ramp: let it
                # carry the second halves of the first two chunks
                eng2 = nc.gpsimd if (edge_opt and ck < 2) else nc.scalar
                nc.sync.dma_start(out=mt[:, 0:h], in_=m[ck][:, 0:h])
                eng2.dma_start(out=mt[:, h:2 * h], in_=m[ck][:, h:2 * h])
                nc.sync.dma_start(out=ft[:, 0:h], in_=f[ck][:, 0:h])
                eng2.dma_start(out=ft[:, h:2 * h], in_=f[ck][:, h:2 * h])
                m_tiles[ck], f_tiles[ck] = mt, ft

            for ck in range(min(PREFETCH + 1, n_chunks)):
                load(ck)
            # pre-warm HAM before the first chunk's matmuls arrive
            for _ in range(prewarm):
                fill_ps = psF.tile([128, CHUNK], F32, tag="fill")
                nc.tensor.matmul(
                    fill_ps, wl_sb[:, 0:128], wt_sb[:, 0:CHUNK],
                    start=True, stop=True,
                )
            for ck in range(n_chunks):
                if ck + PREFETCH + 1 < n_chunks:
                    load(ck + PREFETCH + 1)
                cs = chunk_sizes[ck]
                h = 2 * cs
                mt, ft = m_tiles.pop(ck), f_tiles.pop(ck)
                ot = opool.tile([128, 4 * CHUNK], BF16, tag="o")
                runs = chunk_runs[ck]
                # route the final chunks' stores to the by-then-idle load
                # queues so the tail drains in parallel
                if last_hwdge and ck >= n_chunks - last_hwdge:
                    st1, st2 = nc.sync, nc.scalar
                else:
                    st1 = st2 = nc.gpsimd
                for c in range(4):
                    ct = 0 if c == 0 else 1
                    y_ps = psY.tile([128, CHUNK], F32, tag="y")
                    nc.tensor.matmul(
                        y_ps[:, 0:cs],
                        wl_sb[:, ct * 128:(ct + 1) * 128],
                        mt[:, c * cs:(c + 1) * cs],
                        start=True,
                        stop=False,
                    )
                    for ri, (off, size, g) in enumerate(runs):
                        nc.tensor.matmul(
                            y_ps[:, off:off + size],
                            wt_sb[:, (ct * NA + g) * 128:(ct * NA + g + 1) * 128],
                            ft[:, c * cs + off:c * cs + off + size],
                            start=False,
                            stop=(ri == len(runs) - 1),
                        )
                    if copy_split and c % 2 == 0:
                        nc.scalar.copy(ot[:, c * cs:(c + 1) * cs], y_ps[:, 0:cs])
                    else:
                        nc.vector.tensor_copy(ot[:, c * cs:(c + 1) * cs], y_ps[:, 0:cs])
                    if c == 1:
                        st1.dma_start(out=out[ck][:, 0:h], in_=ot[:, 0:h])
                # fillers: dep-free matmuls on a scratch bank that run in the
                # tail-of-chunk PE idle, keeping HAM activity windows above
                # the K=4/8 demotion threshold
                for _ in range(fillers):
                    fill_ps = psF.tile([128, fillw], F32, tag="fill")
                    nc.tensor.matmul(
                        fill_ps, wl_sb[:, 0:128], wt_sb[:, 0:fillw],
                        start=True, stop=True,
                    )
                st2.dma_start(out=out[ck][:, h:2 * h], in_=ot[:, h:2 * h])

    nc.finalize()
    return nc


def build_program_dense(n_tiles, use_f32r=True):
    """Fallback for non-one-hot attrs: dense sum over the NA attr channels.

    Extra input: att [S, NA]. f32 node-major layout with on-chip PE
    transposes (slow but correct for arbitrary attrs).
    """
    assert n_tiles % 4 == 0
    S = n_tiles * 128
    nc = bacc.Bacc("TRN2")
    m = nc.dram_tensor("m", [S, DIM], F32, kind="ExternalInput")
    f = nc.dram_tensor("f", [S, DIM], F32, kind="ExternalInput")
    att = nc.dram_tensor("att", [S, NA], F32, kind="ExternalInput")
    wl = nc.dram_tensor("wl", [MUL, 2 * MUL], F32, kind="ExternalInput")
    wt = nc.dram_tensor("wt", [MUL, 2 * NA * MUL], F32, kind="ExternalInput")
    ident = nc.dram_tensor("ident", [MUL, MUL], F32, kind="ExternalInput")
    out = nc.dram_tensor("out", [S, DIM], F32, kind="ExternalOutput")

    mm_dt = F32R if use_f32r else F32

    with TileContext(nc) as tc:
        with (
            tc.tile_pool(name="const", bufs=1) as cpool,
            tc.tile_pool(name="io", bufs=6) as iopool,
            tc.tile_pool(name="gsb", bufs=44) as gpool,
            tc.tile_pool(name="tmp", bufs=3) as tmpool,
            tc.tile_pool(name="tsb", bufs=12) as tpool,
            tc.tile_pool(name="ysb", bufs=6) as ypool,
            tc.tile_pool(name="psA", bufs=2, space="PSUM") as psA,
            tc.tile_pool(name="psB", bufs=2, space="PSUM") as psB,
            tc.tile_pool(name="psY", bufs=2, space="PSUM") as psY,
            tc.tile_pool(name="psO", bufs=2, space="PSUM") as psO,
        ):
            wl_sb = cpool.tile([MUL, 2 * MUL], F32, tag="wl")
            nc.sync.dma_start(out=wl_sb, in_=wl[:])
            wt_sb = cpool.tile([MUL, 2 * NA * MUL], F32, tag="wt")
            nc.sync.dma_start(out=wt_sb, in_=wt[:])
            id_sb = cpool.tile([MUL, MUL], F32, tag="ident")
            nc.sync.dma_start(out=id_sb, in_=ident[:])
            if use_f32r:
                wlr = cpool.tile([MUL, 2 * MUL], mm_dt, tag="wlr")
                nc.vector.tensor_copy(wlr, wl_sb)
                wtr = cpool.tile([MUL, 2 * NA * MUL], mm_dt, tag="wtr")
                nc.vector.tensor_copy(wtr, wt_sb)
                wl_sb, wt_sb = wlr, wtr

            for ck in range(n_tiles // 4):
                t0 = ck * 4
                m_tiles, f_tiles, a_tiles = [], [], []
                for nb in range(4):
                    mt = iopool.tile([128, DIM], F32, tag="m_sb")
                    nc.sync.dma_start(out=mt, in_=m[(t0 + nb) * 128:(t0 + nb + 1) * 128])
                    m_tiles.append(mt)
                    ft = iopool.tile([128, DIM], F32, tag="f_sb")
                    nc.sync.dma_start(out=ft, in_=f[(t0 + nb) * 128:(t0 + nb + 1) * 128])
                    f_tiles.append(ft)
                    at = iopool.tile([128, NA], F32, tag="a_sb")
                    nc.sync.dma_start(out=at, in_=att[(t0 + nb) * 128:(t0 + nb + 1) * 128])
                    a_tiles.append(at)

                # pre-scale: g[a][nb] = f[nb] * att[:, a]
                g_tiles = []
                for a in range(NA):
                    row = []
                    for nb in range(4):
                        gt = gpool.tile([128, DIM], F32, tag="g_sb")
                        nc.vector.tensor_scalar_mul(gt, f_tiles[nb], a_tiles[nb][:, a:a + 1])
                        row.append(gt)
                    g_tiles.append(row)

                y_sbs = []
                for c in range(4):
                    ct = 0 if c == 0 else 1
                    tm_ps = psA.tile([128, 512], F32, tag="tm_ps")
                    for nb in range(4):
                        nc.tensor.matmul(
                            tm_ps[:, nb * 128:(nb + 1) * 128],
                            m_tiles[nb][:, c * 128:(c + 1) * 128],
                            id_sb, is_transpose=True,
                        )
                    tm_sb = tmpool.tile([128, 512], mm_dt, tag="tm_sb")
                    nc.scalar.copy(tm_sb, tm_ps)

                    tg_sbs = []
                    for a in range(NA):
                        tg_ps = psB.tile([128, 512], F32, tag="tg_ps")
                        for nb in range(4):
                            nc.tensor.matmul(
                                tg_ps[:, nb * 128:(nb + 1) * 128],
                                g_tiles[a][nb][:, c * 128:(c + 1) * 128],
                                id_sb, is_transpose=True,
                            )
                        tg_sb = tpool.tile([128, 512], mm_dt, tag="tg_sb")
                        if a % 2 == 0:
                            nc.scalar.copy(tg_sb, tg_ps)
                        else:
                            nc.vector.tensor_copy(tg_sb, tg_ps)
                        tg_sbs.append(tg_sb)
                    y_ps = psY.tile([128, 512], F32, tag="y_ps")
                    nc.tensor.matmul(
                        y_ps,
                        wl_sb[:, ct * 128:(ct + 1) * 128],
                        tm_sb,
                        start=True, stop=False,
                    )
                    for a in range(NA):
                        nc.tensor.matmul(
                            y_ps,
                            wt_sb[:, (ct * NA + a) * 128:(ct * NA + a + 1) * 128],
                            tg_sbs[a],
                            start=False, stop=(a == NA - 1),
                        )
                    y_sb = ypool.tile([128, 512], F32, tag="y_sb")
                    if c % 2 == 0:
                        nc.scalar.copy(y_sb, y_ps)
                    else:
                        nc.vector.tensor_copy(y_sb, y_ps)
                    y_sbs.append(y_sb)

                for nb in range(4):
                    o_ps = psO.tile([128, 512], F32, tag="o_ps")
                    for c in range(4):
                        nc.tensor.matmul(
                            o_ps[:, c * 128:(c + 1) * 128],
                            y_sbs[c][:, nb * 128:(nb + 1) * 128],
                            id_sb, is_transpose=True,
                        )
                    o_sb = iopool.tile([128, DIM], F32, tag="o_sb")
                    if nb % 2 == 0:
                        nc.scalar.copy(o_sb, o_ps)
                    else:
                        nc.vector.tensor_copy(o_sb, o_ps)
                    nc.sync.dma_start(
                        out=out[(t0 + nb) * 128:(t0 + nb + 1) * 128], in_=o_sb
                    )

    nc.finalize()
    return nc


def pack_weights(Wl0, Wl1, Wt0, Wt1, np_dt):
    wl = np.concatenate([Wl0 * LIN_SCALE, Wl1 * LIN_SCALE], axis=1).astype(np_dt)
    blocks = [Wt0[:, a, :] * TP_SCALE for a in range(NA)] + [
        Wt1[:, a, :] * TP_SCALE for a in range(NA)
    ]
    wt = np.concatenate(blocks, axis=1).astype(np_dt)
    return np.ascontiguousarray(wl), np.ascontiguousarray(wt)


def plan_grouped(node_attrs):
    """One-hot grouping/sharding plan, or None if attrs are not one-hot.

    Each attr group is split evenly over the cores and padded (per core)
    to the max per-core share so the same program runs on every core;
    group boundaries may fall anywhere inside a chunk (matmul runs use
    arbitrary free-dim offsets).
    """
    N = node_attrs.shape[0]
    z = np.argmax(node_attrs, axis=1)
    onehot = np.zeros_like(node_attrs)
    onehot[np.arange(N), z] = 1.0
    if not np.array_equal(node_attrs, onehot):
        return None

    order = np.argsort(z, kind="stable")
    counts = np.bincount(z, minlength=NA)
    k = -(-counts // N_CORES)  # ceil: per-core padded group size
    S0 = int(k.sum())
    nfull = S0 // CHUNK
    rem = S0 - nfull * CHUNK
    chunk_sizes = [CHUNK] * nfull
    if rem:
        chunk_sizes.append(-(-rem // 32) * 32)  # ragged tail chunk
    S = int(sum(chunk_sizes))
    goff = np.concatenate([[0], np.cumsum(k)])

    # group intervals covering [0, S): extend the last non-empty group
    intervals = [(int(goff[a]), int(goff[a + 1]), a) for a in range(NA) if k[a] > 0]
    lo, hi, g = intervals[-1]
    intervals[-1] = (lo, S, g)

    chunk_runs = []
    c0 = 0
    for cs in chunk_sizes:
        c1 = c0 + cs
        runs = []
        for lo, hi, g in intervals:
            s, e = max(lo, c0), min(hi, c1)
            if s < e:
                runs.append((s - c0, e - s, g))
        chunk_runs.append(tuple(runs))
        c0 = c1

    per_core_idx = [[] for _ in range(N_CORES)]
    per_core_pos = [[] for _ in range(N_CORES)]
    pos = 0
    for a in range(NA):
        ga = order[pos:pos + counts[a]]
        pos += counts[a]
        q, r = divmod(len(ga), N_CORES)
        off = 0
        for cidx in range(N_CORES):
            take = q + (1 if cidx < r else 0)
            per_core_idx[cidx].append(ga[off:off + take])
            per_core_pos[cidx].append(goff[a] + np.arange(take))
            off += take

    plans = []
    for cidx in range(N_CORES):
        idx = np.concatenate(per_core_idx[cidx])
        posn = np.concatenate(per_core_pos[cidx]).astype(np.int64)
        plans.append((idx, posn))
    return dict(S=S, chunk_sizes=tuple(chunk_sizes),
                chunk_runs=tuple(chunk_runs), plans=plans)


def pack_tiled(rows_bf16, posn, chunk_sizes):
    """[n_rows, 512] bf16 (plane-major cols) -> [n_chunks, 128, 2048] tiled.

    Full chunks hold [128, 4, 512]; a ragged last chunk is packed at
    [128, 4, cs] within the leading 4*cs cols of its block.
    """
    S = int(sum(chunk_sizes))
    n_chunks = len(chunk_sizes)
    pad = np.zeros((S, DIM), dtype=NP_BF16)
    pad[posn] = rows_bf16
    buf = np.zeros((n_chunks, 128, 4 * CHUNK), dtype=NP_BF16)
    nfull = sum(1 for cs in chunk_sizes if cs == CHUNK)
    if nfull:
        t = pad[:nfull * CHUNK].reshape(nfull, CHUNK, 4, 128).transpose(0, 3, 2, 1)
        buf[:nfull] = np.ascontiguousarray(t).reshape(nfull, 128, 4 * CHUNK)
    if nfull < n_chunks:
        cs = chunk_sizes[-1]
        t = pad[nfull * CHUNK:].reshape(cs, 4, 128).transpose(2, 1, 0)
        buf[-1][:, :4 * cs] = np.ascontiguousarray(t).reshape(128, 4 * cs)
    return buf


def unpack_tiled(tiled, posn, chunk_sizes):
    """[n_chunks, 128, 2048] bf16 -> rows [len(posn), 512] bf16 plane-major."""
    S = int(sum(chunk_sizes))
    n_chunks = len(chunk_sizes)
    nfull = sum(1 for cs in chunk_sizes if cs == CHUNK)
    y = np.empty((S, DIM), dtype=NP_BF16)
    if nfull:
        t = tiled[:nfull].reshape(nfull, 128, 4, CHUNK).transpose(0, 3, 2, 1)
        y[:nfull * CHUNK] = np.ascontiguousarray(t).reshape(nfull * CHUNK, DIM)
    if nfull < n_chunks:
        cs = chunk_sizes[-1]
        t = tiled[-1][:, :4 * cs].reshape(128, 4, cs).transpose(2, 1, 0)
        y[nfull * CHUNK:] = np.ascontiguousarray(t).reshape(cs, DIM)
    return y[posn]


_CACHE = {}


def kernel(m_i, node_feats, node_attrs, Wl0, Wl1, Wt0, Wt1):
    global LAST_RESULTS
    import os
    trace = bool(os.environ.get("KERNEL_TRACE"))
    m_i = np.ascontiguousarray(m_i, dtype=np.float32)
    node_feats = np.ascontiguousarray(node_feats, dtype=np.float32)
    node_attrs = np.ascontiguousarray(node_attrs, dtype=np.float32)
    N = m_i.shape[0]

    plan = plan_grouped(node_attrs)
    if plan is not None:
        wl, wt = pack_weights(Wl0, Wl1, Wt0, Wt1, NP_BF16)
        key = ("grouped", plan["chunk_sizes"], plan["chunk_runs"])
        if key not in _CACHE:
            _CACHE.clear()
            _CACHE[key] = build_program(plan["chunk_sizes"], plan["chunk_runs"])
        nc = _CACHE[key]
        chunk_sizes = plan["chunk_sizes"]
        mp = m_i.astype(NP_BF16)[:, COL_PERM]
        fp = node_feats.astype(NP_BF16)[:, COL_PERM]
        in_maps = []
        for cidx in range(N_CORES):
            idx, posn = plan["plans"][cidx]
            in_maps.append(dict(
                m=pack_tiled(mp[idx], posn, chunk_sizes),
                f=pack_tiled(fp[idx], posn, chunk_sizes),
                wl=wl, wt=wt,
            ))
        res = run_bass_kernel_spmd(
            nc, in_maps, core_ids=list(range(N_CORES)), trace=trace
        )
        LAST_RESULTS = res
        out = np.empty((N, DIM), dtype=np.float32)
        for cidx in range(N_CORES):
            idx, posn = plan["plans"][cidx]
            rows = unpack_tiled(res.results[cidx]["out"], posn, chunk_sizes)
            out[idx] = rows.astype(np.float32)[:, COL_PERM_INV]
        return out

    # dense fallback (arbitrary attrs)
    wl, wt = pack_weights(Wl0, Wl1, Wt0, Wt1, np.float32)
    ident = np.eye(128, dtype=np.float32)
    mp = np.ascontiguousarray(m_i[:, COL_PERM])
    fp = np.ascontiguousarray(node_feats[:, COL_PERM])
    per_core = max(512, int(np.ceil(N / N_CORES / 512.0)) * 512)
    S = per_core
    key = ("dense", S)
    if key not in _CACHE:
        _CACHE.clear()
        _CACHE[key] = build_program_dense(S // 128)
    nc = _CACHE[key]
    in_maps = []
    bounds = []
    for cidx in range(N_CORES):
        lo = min(cidx * per_core, N)
        hi = min(lo + per_core, N)
        mpad = np.zeros((S, DIM), dtype=np.float32)
        fpad = np.zeros((S, DIM), dtype=np.float32)
        apad = np.zeros((S, NA), dtype=np.float32)
        mpad[:hi - lo] = mp[lo:hi]
        fpad[:hi - lo] = fp[lo:hi]
        apad[:hi - lo] = node_attrs[lo:hi]
        in_maps.append(dict(m=mpad, f=fpad, att=apad, wl=wl, wt=wt, ident=ident))
        bounds.append((lo, hi))
    res = run_bass_kernel_spmd(
        nc, in_maps, core_ids=list(range(N_CORES)), trace=trace
    )
    LAST_RESULTS = res
    out = np.empty((N, DIM), dtype=np.float32)
    for cidx, (lo, hi) in enumerate(bounds):
        out[lo:hi] = res.results[cidx]["out"][:hi - lo]
    return np.ascontiguousarray(out[:, COL_PERM_INV])


# revision 25
# speedup vs baseline: 1.1278x; 1.1278x over previous
"""Trainium2 (8-core) kernel for nn_NodeUpdateBlock: equivariant Linear +
FullyConnectedTensorProduct with 10 scalar (0e) one-hot attributes.

Self-contained: takes FULL inputs (as produced by the problem's
setup_inputs), distributes across the 8 NeuronCores internally, and
returns the FULL [N, 512] float32 output.

Strategy
--------
out_c = m_c @ (Wl_c * ls) + sum_a (att_a * f_c) @ (Wt_c[:,a,:] * ts)
per irrep-component plane c (1 scalar plane + 3 vector planes of 128
channels).  node_attrs rows are one-hot, so on the host we compute
z = argmax(attrs), sort nodes by z, and split every attr-group evenly
over the 8 cores; the tensor product then needs exactly ONE matmul per
(plane, group-run) with a compile-time weight slice.

The kernel is memory-bound, so the data plane is bf16: the host
downcasts m/f (and the weights) to bf16, transposes to channel-major,
and pre-tiles into the exact SBUF layout [chunk, 128 part, plane, 512
nodes] so every DMA packet is a contiguous 4 KiB run on both sides and
the PE needs no on-chip transposes at all.  Per 512-node chunk and
plane, one PSUM accumulation group computes
Y^T = WL^T m^T + WT_g^T f^T, and the result is copied to SBUF as bf16
and streamed out.  The host un-tiles and upcasts to f32 (bf16 rel-err
~3e-3, well inside the 2e-2 gate).  A dense (non-one-hot) f32 fallback
sums over all 10 attribute channels and stays correct for arbitrary
node_attrs.
"""

import math

import numpy as np
import ml_dtypes

import concourse.bacc as bacc
import concourse.mybir as mybir
from concourse.tile import TileContext
from concourse.bass_utils import run_bass_kernel_spmd

MUL = 128
NA = 10
DIM = 512
N_CORES = 8
CHUNK = 512  # nodes per compute chunk
LIN_SCALE = 1.0 / math.sqrt(MUL)
TP_SCALE = 1.0 / math.sqrt(MUL * NA)
F32 = mybir.dt.float32
F32R = mybir.dt.float32r
BF16 = mybir.dt.bfloat16
NP_BF16 = ml_dtypes.bfloat16

LAST_RESULTS = None  # BassKernelResults of the most recent run (for testing)


def _col_perm():
    perm = list(range(MUL))
    for x in range(3):
        perm += [MUL + 3 * i + x for i in range(MUL)]
    return np.array(perm, dtype=np.int64)


COL_PERM = _col_perm()
COL_PERM_INV = np.argsort(COL_PERM)


def build_program(chunk_sizes, chunk_runs, edge_opt=False, fillers=3, prefetch=4, copy_split=False, prewarm=0, last_hwdge=2, fillw=CHUNK, psy=7, early_gpl=0, qlast=False):
    """Bass program for one core (same program runs on all cores).

    Inputs:  m/f [n_chunks, 128, 2048] bf16 (pre-tiled channel-major; the
             last chunk may be ragged, packed at 4*cs cols within its block),
             wl [128, 256] bf16, wt [128, 2560] bf16
    Output:  out [n_chunks, 128, 2048] bf16 (same tiling)
    """
    n_chunks = len(chunk_sizes)
    nc = bacc.Bacc("TRN2")
    m = nc.dram_tensor("m", [n_chunks, 128, 4 * CHUNK], BF16, kind="ExternalInput")
    f = nc.dram_tensor("f", [n_chunks, 128, 4 * CHUNK], BF16, kind="ExternalInput")
    wl = nc.dram_tensor("wl", [MUL, 2 * MUL], BF16, kind="ExternalInput")
    wt = nc.dram_tensor("wt", [MUL, 2 * NA * MUL], BF16, kind="ExternalInput")
    out = nc.dram_tensor("out", [n_chunks, 128, 4 * CHUNK], BF16, kind="ExternalOutput")

    with TileContext(nc) as tc:
        with (
            tc.tile_pool(name="const", bufs=1) as cpool,
            tc.tile_pool(name="mio", bufs=7) as mpool,
            tc.tile_pool(name="fio", bufs=7) as fpool,
            tc.tile_pool(name="oio", bufs=6) as opool,
            tc.tile_pool(name="psY", bufs=psy, space="PSUM") as psY,
            tc.tile_pool(name="psF", bufs=1, space="PSUM") as psF,
        ):
            # weights ride the (otherwise idle at t=0) gpsimd queue so the
            # sync/scalar queues start streaming chunk data immediately
            wl_sb = cpool.tile([MUL, 2 * MUL], BF16, tag="wl")
            nc.gpsimd.dma_start(out=wl_sb, in_=wl[:])
            wt_sb = cpool.tile([MUL, 2 * NA * MUL], BF16, tag="wt")
            nc.gpsimd.dma_start(out=wt_sb, in_=wt[:])

            # loads split half/half across the two HWDGE queues; stores on
            # gpsimd.  Loads are issued PREFETCH chunks ahead so their
            # issue slots never queue behind PSUM-copy waits on the same
            # engine.
            PREFETCH = prefetch
            m_tiles, f_tiles = {}, {}

            def load(ck):
                cs = chunk_sizes[ck]
                h = 2 * cs
                mt = mpool.tile([128, 4 * CHUNK], BF16, tag="m")
                ft = fpool.tile([128, 4 * CHUNK], BF16, tag="f")
                # the store queue (gpsimd) is idle during the ramp: let it
                # carry the second halves of the first two chunks
                eng2 = nc.gpsimd if ck < early_gpl else nc.scalar
                nc.sync.dma_start(out=mt[:, 0:h], in_=m[ck][:, 0:h])
                eng2.dma_start(out=mt[:, h:2 * h], in_=m[ck][:, h:2 * h])
                nc.sync.dma_start(out=ft[:, 0:h], in_=f[ck][:, 0:h])
                eng2.dma_start(out=ft[:, h:2 * h], in_=f[ck][:, h:2 * h])
                m_tiles[ck], f_tiles[ck] = mt, ft

            for ck in range(min(PREFETCH + 1, n_chunks)):
                load(ck)
            # pre-warm HAM before the first chunk's matmuls arrive
            for _ in range(prewarm):
                fill_ps = psF.tile([128, CHUNK], F32, tag="fill")
                nc.tensor.matmul(
                    fill_ps, wl_sb[:, 0:128], wt_sb[:, 0:CHUNK],
                    start=True, stop=True,
                )
            for ck in range(n_chunks):
                if ck + PREFETCH + 1 < n_chunks:
                    load(ck + PREFETCH + 1)
                cs = chunk_sizes[ck]
                h = 2 * cs
                mt, ft = m_tiles.pop(ck), f_tiles.pop(ck)
                ot = opool.tile([128, 4 * CHUNK], BF16, tag="o")
                runs = chunk_runs[ck]
                # route the final chunks' stores to the by-then-idle load
                # queues so the tail drains in parallel
                if last_hwdge and ck >= n_chunks - last_hwdge:
                    st1, st2 = nc.sync, nc.scalar
                else:
                    st1 = st2 = nc.gpsimd
                for c in range(4):
                    ct = 0 if c == 0 else 1
                    y_ps = psY.tile([128, CHUNK], F32, tag="y")
                    nc.tensor.matmul(
                        y_ps[:, 0:cs],
                        wl_sb[:, ct * 128:(ct + 1) * 128],
                        mt[:, c * cs:(c + 1) * cs],
                        start=True,
                        stop=False,
                    )
                    for ri, (off, size, g) in enumerate(runs):
                        nc.tensor.matmul(
                            y_ps[:, off:off + size],
                            wt_sb[:, (ct * NA + g) * 128:(ct * NA + g + 1) * 128],
                            ft[:, c * cs + off:c * cs + off + size],
                            start=False,
                            stop=(ri == len(runs) - 1),
                        )
                    if copy_split and c % 2 == 0:
                        nc.scalar.copy(ot[:, c * cs:(c + 1) * cs], y_ps[:, 0:cs])
                    else:
                        nc.vector.tensor_copy(ot[:, c * cs:(c + 1) * cs], y_ps[:, 0:cs])
                    if c == 1:
                        st1.dma_start(out=out[ck][:, 0:h], in_=ot[:, 0:h])
                # fillers: dep-free matmuls on a scratch bank that run in the
                # tail-of-chunk PE idle, keeping HAM activity windows above
                # the K=4/8 demotion threshold
                for _ in range(fillers):
                    fill_ps = psF.tile([128, fillw], F32, tag="fill")
                    nc.tensor.matmul(
                        fill_ps, wl_sb[:, 0:128], wt_sb[:, 0:fillw],
                        start=True, stop=True,
                    )
                if qlast and ck == n_chunks - 1:
                    # split the very last half-store across two queues
                    q = h // 2
                    nc.scalar.dma_start(out=out[ck][:, h:h + q], in_=ot[:, h:h + q])
                    nc.gpsimd.dma_start(out=out[ck][:, h + q:2 * h], in_=ot[:, h + q:2 * h])
                else:
                    st2.dma_start(out=out[ck][:, h:2 * h], in_=ot[:, h:2 * h])

    nc.finalize()
    return nc


def build_program_dense(n_tiles, use_f32r=True):
    """Fallback for non-one-hot attrs: dense sum over the NA attr channels.

    Extra input: att [S, NA]. f32 node-major layout with on-chip PE
    transposes (slow but correct for arbitrary attrs).
    """
    assert n_tiles % 4 == 0
    S = n_tiles * 128
    nc = bacc.Bacc("TRN2")
    m = nc.dram_tensor("m", [S, DIM], F32, kind="ExternalInput")
    f = nc.dram_tensor("f", [S, DIM], F32, kind="ExternalInput")
    att = nc.dram_tensor("att", [S, NA], F32, kind="ExternalInput")
    wl = nc.dram_tensor("wl", [MUL, 2 * MUL], F32, kind="ExternalInput")
    wt = nc.dram_tensor("wt", [MUL, 2 * NA * MUL], F32, kind="ExternalInput")
    ident = nc.dram_tensor("ident", [MUL, MUL], F32, kind="ExternalInput")
    out = nc.dram_tensor("out", [S, DIM], F32, kind="ExternalOutput")

    mm_dt = F32R if use_f32r else F32

    with TileContext(nc) as tc:
        with (
            tc.tile_pool(name="const", bufs=1) as cpool,
            tc.tile_pool(name="io", bufs=6) as iopool,
            tc.tile_pool(name="gsb", bufs=44) as gpool,
            tc.tile_pool(name="tmp", bufs=3) as tmpool,
            tc.tile_pool(name="tsb", bufs=12) as tpool,
            tc.tile_pool(name="ysb", bufs=6) as ypool,
            tc.tile_pool(name="psA", bufs=2, space="PSUM") as psA,
            tc.tile_pool(name="psB", bufs=2, space="PSUM") as psB,
            tc.tile_pool(name="psY", bufs=2, space="PSUM") as psY,
            tc.tile_pool(name="psO", bufs=2, space="PSUM") as psO,
        ):
            wl_sb = cpool.tile([MUL, 2 * MUL], F32, tag="wl")
            nc.sync.dma_start(out=wl_sb, in_=wl[:])
            wt_sb = cpool.tile([MUL, 2 * NA * MUL], F32, tag="wt")
            nc.sync.dma_start(out=wt_sb, in_=wt[:])
            id_sb = cpool.tile([MUL, MUL], F32, tag="ident")
            nc.sync.dma_start(out=id_sb, in_=ident[:])
            if use_f32r:
                wlr = cpool.tile([MUL, 2 * MUL], mm_dt, tag="wlr")
                nc.vector.tensor_copy(wlr, wl_sb)
                wtr = cpool.tile([MUL, 2 * NA * MUL], mm_dt, tag="wtr")
                nc.vector.tensor_copy(wtr, wt_sb)
                wl_sb, wt_sb = wlr, wtr

            for ck in range(n_tiles // 4):
                t0 = ck * 4
                m_tiles, f_tiles, a_tiles = [], [], []
                for nb in range(4):
                    mt = iopool.tile([128, DIM], F32, tag="m_sb")
                    nc.sync.dma_start(out=mt, in_=m[(t0 + nb) * 128:(t0 + nb + 1) * 128])
                    m_tiles.append(mt)
                    ft = iopool.tile([128, DIM], F32, tag="f_sb")
                    nc.sync.dma_start(out=ft, in_=f[(t0 + nb) * 128:(t0 + nb + 1) * 128])
                    f_tiles.append(ft)
                    at = iopool.tile([128, NA], F32, tag="a_sb")
                    nc.sync.dma_start(out=at, in_=att[(t0 + nb) * 128:(t0 + nb + 1) * 128])
                    a_tiles.append(at)

                # pre-scale: g[a][nb] = f[nb] * att[:, a]
                g_tiles = []
                for a in range(NA):
                    row = []
                    for nb in range(4):
                        gt = gpool.tile([128, DIM], F32, tag="g_sb")
                        nc.vector.tensor_scalar_mul(gt, f_tiles[nb], a_tiles[nb][:, a:a + 1])
                        row.append(gt)
                    g_tiles.append(row)

                y_sbs = []
                for c in range(4):
                    ct = 0 if c == 0 else 1
                    tm_ps = psA.tile([128, 512], F32, tag="tm_ps")
                    for nb in range(4):
                        nc.tensor.matmul(
                            tm_ps[:, nb * 128:(nb + 1) * 128],
                            m_tiles[nb][:, c * 128:(c + 1) * 128],
                            id_sb, is_transpose=True,
                        )
                    tm_sb = tmpool.tile([128, 512], mm_dt, tag="tm_sb")
                    nc.scalar.copy(tm_sb, tm_ps)

                    tg_sbs = []
                    for a in range(NA):
                        tg_ps = psB.tile([128, 512], F32, tag="tg_ps")
                        for nb in range(4):
                            nc.tensor.matmul(
                                tg_ps[:, nb * 128:(nb + 1) * 128],
                                g_tiles[a][nb][:, c * 128:(c + 1) * 128],
                                id_sb, is_transpose=True,
                            )
                        tg_sb = tpool.tile([128, 512], mm_dt, tag="tg_sb")
                        if a % 2 == 0:
                            nc.scalar.copy(tg_sb, tg_ps)
                        else:
                            nc.vector.tensor_copy(tg_sb, tg_ps)
                        tg_sbs.append(tg_sb)
                    y_ps = psY.tile([128, 512], F32, tag="y_ps")
                    nc.tensor.matmul(
                        y_ps,
                        wl_sb[:, ct * 128:(ct + 1) * 128],
                        tm_sb,
                        start=True, stop=False,
                    )
                    for a in range(NA):
                        nc.tensor.matmul(
                            y_ps,
                            wt_sb[:, (ct * NA + a) * 128:(ct * NA + a + 1) * 128],
                            tg_sbs[a],
                            start=False, stop=(a == NA - 1),
                        )
                    y_sb = ypool.tile([128, 512], F32, tag="y_sb")
                    if c % 2 == 0:
                        nc.scalar.copy(y_sb, y_ps)
                    else:
                        nc.vector.tensor_copy(y_sb, y_ps)
                    y_sbs.append(y_sb)

                for nb in range(4):
                    o_ps = psO.tile([128, 512], F32, tag="o_ps")
                    for c in range(4):
                        nc.tensor.matmul(
                            o_ps[:, c * 128:(c + 1) * 128],
                            y_sbs[c][:, nb * 128:(nb + 1) * 128],
                            id_sb, is_transpose=True,
                        )
                    o_sb = iopool.tile([128, DIM], F32, tag="o_sb")
                    if nb % 2 == 0:
                        nc.scalar.copy(o_sb, o_ps)
                    else:
                        nc.vector.tensor_copy(o_sb, o_ps)
                    nc.sync.dma_start(
                        out=out[(t0 + nb) * 128:(t0 + nb + 1) * 128], in_=o_sb
                    )

    nc.finalize()
    return nc


def pack_weights(Wl0, Wl1, Wt0, Wt1, np_dt):
    wl = np.concatenate([Wl0 * LIN_SCALE, Wl1 * LIN_SCALE], axis=1).astype(np_dt)
    blocks = [Wt0[:, a, :] * TP_SCALE for a in range(NA)] + [
        Wt1[:, a, :] * TP_SCALE for a in range(NA)
    ]
    wt = np.concatenate(blocks, axis=1).astype(np_dt)
    return np.ascontiguousarray(wl), np.ascontiguousarray(wt)


def plan_grouped(node_attrs):
    """One-hot grouping/sharding plan, or None if attrs are not one-hot.

    Each attr group is split evenly over the cores and padded (per core)
    to the max per-core share so the same program runs on every core;
    group boundaries may fall anywhere inside a chunk (matmul runs use
    arbitrary free-dim offsets).
    """
    N = node_attrs.shape[0]
    z = np.argmax(node_attrs, axis=1)
    onehot = np.zeros_like(node_attrs)
    onehot[np.arange(N), z] = 1.0
    if not np.array_equal(node_attrs, onehot):
        return None

    order = np.argsort(z, kind="stable")
    counts = np.bincount(z, minlength=NA)
    k = -(-counts // N_CORES)  # ceil: per-core padded group size
    S0 = int(k.sum())
    nfull = S0 // CHUNK
    rem = S0 - nfull * CHUNK
    chunk_sizes = [CHUNK] * nfull
    if rem:
        chunk_sizes.append(-(-rem // 32) * 32)  # ragged tail chunk
    S = int(sum(chunk_sizes))
    goff = np.concatenate([[0], np.cumsum(k)])

    # group intervals covering [0, S): extend the last non-empty group
    intervals = [(int(goff[a]), int(goff[a + 1]), a) for a in range(NA) if k[a] > 0]
    lo, hi, g = intervals[-1]
    intervals[-1] = (lo, S, g)

    chunk_runs = []
    c0 = 0
    for cs in chunk_sizes:
        c1 = c0 + cs
        runs = []
        for lo, hi, g in intervals:
            s, e = max(lo, c0), min(hi, c1)
            if s < e:
                runs.append((s - c0, e - s, g))
        chunk_runs.append(tuple(runs))
        c0 = c1

    per_core_idx = [[] for _ in range(N_CORES)]
    per_core_pos = [[] for _ in range(N_CORES)]
    pos = 0
    for a in range(NA):
        ga = order[pos:pos + counts[a]]
        pos += counts[a]
        q, r = divmod(len(ga), N_CORES)
        off = 0
        for cidx in range(N_CORES):
            take = q + (1 if cidx < r else 0)
            per_core_idx[cidx].append(ga[off:off + take])
            per_core_pos[cidx].append(goff[a] + np.arange(take))
            off += take

    plans = []
    for cidx in range(N_CORES):
        idx = np.concatenate(per_core_idx[cidx])
        posn = np.concatenate(per_core_pos[cidx]).astype(np.int64)
        plans.append((idx, posn))
    return dict(S=S, chunk_sizes=tuple(chunk_sizes),
                chunk_runs=tuple(chunk_runs), plans=plans)


def pack_tiled(rows_bf16, posn, chunk_sizes):
    """[n_rows, 512] bf16 (plane-major cols) -> [n_chunks, 128, 2048] tiled.

    Full chunks hold [128, 4, 512]; a ragged last chunk is packed at
    [128, 4, cs] within the leading 4*cs cols of its block.
    """
    S = int(sum(chunk_sizes))
    n_chunks = len(chunk_sizes)
    pad = np.zeros((S, DIM), dtype=NP_BF16)
    pad[posn] = rows_bf16
    buf = np.zeros((n_chunks, 128, 4 * CHUNK), dtype=NP_BF16)
    nfull = sum(1 for cs in chunk_sizes if cs == CHUNK)
    if nfull:
        t = pad[:nfull * CHUNK].reshape(nfull, CHUNK, 4, 128).transpose(0, 3, 2, 1)
        buf[:nfull] = np.ascontiguousarray(t).reshape(nfull, 128, 4 * CHUNK)
    if nfull < n_chunks:
        cs = chunk_sizes[-1]
        t = pad[nfull * CHUNK:].reshape(cs, 4, 128).transpose(2, 1, 0)
        buf[-1][:, :4 * cs] = np.ascontiguousarray(t).reshape(128, 4 * cs)
    return buf


def unpack_tiled(tiled, posn, chunk_sizes):
    """[n_chunks, 128, 2048] bf16 -> rows [len(posn), 512] bf16 plane-major."""
    S = int(sum(chunk_sizes))
    n_chunks = len(chunk_sizes)
    nfull = sum(1 for cs in chunk_sizes if cs == CHUNK)
    y = np.empty((S, DIM), dtype=NP_BF16)
    if nfull:
        t = tiled[:nfull].reshape(nfull, 128, 4, CHUNK).transpose(0, 3, 2, 1)
        y[:nfull * CHUNK] = np.ascontiguousarray(t).reshape(nfull * CHUNK, DIM)
    if nfull < n_chunks:
        cs = chunk_sizes[-1]
        t = tiled[-1][:, :4 * cs].reshape(128, 4, cs).transpose(2, 1, 0)
        y[nfull * CHUNK:] = np.ascontiguousarray(t).reshape(cs, DIM)
    return y[posn]


_CACHE = {}


def kernel(m_i, node_feats, node_attrs, Wl0, Wl1, Wt0, Wt1):
    global LAST_RESULTS
    import os
    trace = bool(os.environ.get("KERNEL_TRACE"))
    m_i = np.ascontiguousarray(m_i, dtype=np.float32)
    node_feats = np.ascontiguousarray(node_feats, dtype=np.float32)
    node_attrs = np.ascontiguousarray(node_attrs, dtype=np.float32)
    N = m_i.shape[0]

    plan = plan_grouped(node_attrs)
    if plan is not None:
        wl, wt = pack_weights(Wl0, Wl1, Wt0, Wt1, NP_BF16)
        key = ("grouped", plan["chunk_sizes"], plan["chunk_runs"])
        if key not in _CACHE:
            _CACHE.clear()
            _CACHE[key] = build_program(plan["chunk_sizes"], plan["chunk_runs"])
        nc = _CACHE[key]
        chunk_sizes = plan["chunk_sizes"]
        mp = m_i.astype(NP_BF16)[:, COL_PERM]
        fp = node_feats.astype(NP_BF16)[:, COL_PERM]
        in_maps = []
        for cidx in range(N_CORES):
            idx, posn = plan["plans"][cidx]
            in_maps.append(dict(
                m=pack_tiled(mp[idx], posn, chunk_sizes),
                f=pack_tiled(fp[idx], posn, chunk_sizes),
                wl=wl, wt=wt,
            ))
        res = run_bass_kernel_spmd(
            nc, in_maps, core_ids=list(range(N_CORES)), trace=trace
        )
        LAST_RESULTS = res
        out = np.empty((N, DIM), dtype=np.float32)
        for cidx in range(N_CORES):
            idx, posn = plan["plans"][cidx]
            rows = unpack_tiled(res.results[cidx]["out"], posn, chunk_sizes)
            out[idx] = rows.astype(np.float32)[:, COL_PERM_INV]
        return out

    # dense fallback (arbitrary attrs)
    wl, wt = pack_weights(Wl0, Wl1, Wt0, Wt1, np.float32)
    ident = np.eye(128, dtype=np.float32)
    mp = np.ascontiguousarray(m_i[:, COL_PERM])
    fp = np.ascontiguousarray(node_feats[:, COL_PERM])
    per_core = max(512, int(np.ceil(N / N_CORES / 512.0)) * 512)
    S = per_core
    key = ("dense", S)
    if key not in _CACHE:
        _CACHE.clear()
        _CACHE[key] = build_program_dense(S // 128)
    nc = _CACHE[key]
    in_maps = []
    bounds = []
    for cidx in range(N_CORES):
        lo = min(cidx * per_core, N)
        hi = min(lo + per_core, N)
        mpad = np.zeros((S, DIM), dtype=np.float32)
        fpad = np.zeros((S, DIM), dtype=np.float32)
        apad = np.zeros((S, NA), dtype=np.float32)
        mpad[:hi - lo] = mp[lo:hi]
        fpad[:hi - lo] = fp[lo:hi]
        apad[:hi - lo] = node_attrs[lo:hi]
        in_maps.append(dict(m=mpad, f=fpad, att=apad, wl=wl, wt=wt, ident=ident))
        bounds.append((lo, hi))
    res = run_bass_kernel_spmd(
        nc, in_maps, core_ids=list(range(N_CORES)), trace=trace
    )
    LAST_RESULTS = res
    out = np.empty((N, DIM), dtype=np.float32)
    for cidx, (lo, hi) in enumerate(bounds):
        out[lo:hi] = res.results[cidx]["out"][:hi - lo]
    return np.ascontiguousarray(out[:, COL_PERM_INV])


# revision 26
# speedup vs baseline: 1.1698x; 1.0372x over previous
"""Trainium2 (8-core) kernel for nn_NodeUpdateBlock: equivariant Linear +
FullyConnectedTensorProduct with 10 scalar (0e) one-hot attributes.

Self-contained: takes FULL inputs (as produced by the problem's
setup_inputs), distributes across the 8 NeuronCores internally, and
returns the FULL [N, 512] float32 output.

Strategy
--------
out_c = m_c @ (Wl_c * ls) + sum_a (att_a * f_c) @ (Wt_c[:,a,:] * ts)
per irrep-component plane c (1 scalar plane + 3 vector planes of 128
channels).  node_attrs rows are one-hot, so on the host we compute
z = argmax(attrs), sort nodes by z, and split every attr-group evenly
over the 8 cores; the tensor product then needs exactly ONE matmul per
(plane, group-run) with a compile-time weight slice.

The kernel is memory-bound, so the data plane is bf16: the host
downcasts m/f (and the weights) to bf16, transposes to channel-major,
and pre-tiles into the exact SBUF layout [chunk, 128 part, plane, 512
nodes] so every DMA packet is a contiguous 4 KiB run on both sides and
the PE needs no on-chip transposes at all.  Per 512-node chunk and
plane, one PSUM accumulation group computes
Y^T = WL^T m^T + WT_g^T f^T, and the result is copied to SBUF as bf16
and streamed out.  The host un-tiles and upcasts to f32 (bf16 rel-err
~3e-3, well inside the 2e-2 gate).  A dense (non-one-hot) f32 fallback
sums over all 10 attribute channels and stays correct for arbitrary
node_attrs.
"""

import math

import numpy as np
import ml_dtypes

import concourse.bacc as bacc
import concourse.mybir as mybir
from concourse.tile import TileContext
from concourse.bass_utils import run_bass_kernel_spmd

MUL = 128
NA = 10
DIM = 512
N_CORES = 8
CHUNK = 512  # nodes per compute chunk
LIN_SCALE = 1.0 / math.sqrt(MUL)
TP_SCALE = 1.0 / math.sqrt(MUL * NA)
F32 = mybir.dt.float32
F32R = mybir.dt.float32r
BF16 = mybir.dt.bfloat16
FP8 = mybir.dt.float8e4
NP_BF16 = ml_dtypes.bfloat16
NP_FP8 = ml_dtypes.float8_e4m3

LAST_RESULTS = None  # BassKernelResults of the most recent run (for testing)


def _col_perm():
    perm = list(range(MUL))
    for x in range(3):
        perm += [MUL + 3 * i + x for i in range(MUL)]
    return np.array(perm, dtype=np.int64)


COL_PERM = _col_perm()
COL_PERM_INV = np.argsort(COL_PERM)


def build_program(chunk_sizes, chunk_runs, edge_opt=False, fillers=3, prefetch=4, copy_split=False, prewarm=0, last_hwdge=2, fillw=CHUNK, psy=7, early_gpl=0, qlast=False):
    """Bass program for one core (same program runs on all cores).

    Inputs:  m/f [n_chunks, 128, 2048] bf16 (pre-tiled channel-major; the
             last chunk may be ragged, packed at 4*cs cols within its block),
             wl [128, 256] bf16, wt [128, 2560] bf16
    Output:  out [n_chunks, 128, 2048] bf16 (same tiling)
    """
    n_chunks = len(chunk_sizes)
    nc = bacc.Bacc("TRN2")
    m = nc.dram_tensor("m", [n_chunks, 128, 4 * CHUNK], BF16, kind="ExternalInput")
    f = nc.dram_tensor("f", [n_chunks, 128, 4 * CHUNK], FP8, kind="ExternalInput")
    wl = nc.dram_tensor("wl", [MUL, 2 * MUL], BF16, kind="ExternalInput")
    wt = nc.dram_tensor("wt", [MUL, 2 * NA * MUL], BF16, kind="ExternalInput")
    out = nc.dram_tensor("out", [n_chunks, 128, 4 * CHUNK], BF16, kind="ExternalOutput")

    with TileContext(nc) as tc:
        with (
            tc.tile_pool(name="const", bufs=1) as cpool,
            tc.tile_pool(name="mio", bufs=7) as mpool,
            tc.tile_pool(name="fio", bufs=7) as fpool,
            tc.tile_pool(name="oio", bufs=6) as opool,
            tc.tile_pool(name="psY", bufs=psy, space="PSUM") as psY,
            tc.tile_pool(name="psF", bufs=1, space="PSUM") as psF,
        ):
            # weights ride the (otherwise idle at t=0) gpsimd queue so the
            # sync/scalar queues start streaming chunk data immediately
            wl_sb = cpool.tile([MUL, 2 * MUL], BF16, tag="wl")
            nc.gpsimd.dma_start(out=wl_sb, in_=wl[:])
            wt_sb = cpool.tile([MUL, 2 * NA * MUL], BF16, tag="wt")
            nc.gpsimd.dma_start(out=wt_sb, in_=wt[:])

            # loads split half/half across the two HWDGE queues; stores on
            # gpsimd.  Loads are issued PREFETCH chunks ahead so their
            # issue slots never queue behind PSUM-copy waits on the same
            # engine.
            PREFETCH = prefetch
            m_tiles, f_tiles = {}, {}

            def load(ck):
                cs = chunk_sizes[ck]
                h = 2 * cs
                mt = mpool.tile([128, 4 * CHUNK], BF16, tag="m")
                ft = fpool.tile([128, 4 * CHUNK], FP8, tag="f")
                # the store queue (gpsimd) is idle during the ramp: let it
                # carry the second halves of the first two chunks
                eng2 = nc.gpsimd if ck < early_gpl else nc.scalar
                nc.sync.dma_start(out=mt[:, 0:h], in_=m[ck][:, 0:h])
                eng2.dma_start(out=mt[:, h:2 * h], in_=m[ck][:, h:2 * h])
                nc.sync.dma_start(out=ft[:, 0:h], in_=f[ck][:, 0:h])
                eng2.dma_start(out=ft[:, h:2 * h], in_=f[ck][:, h:2 * h])
                m_tiles[ck], f_tiles[ck] = mt, ft

            for ck in range(min(PREFETCH + 1, n_chunks)):
                load(ck)
            # pre-warm HAM before the first chunk's matmuls arrive
            for _ in range(prewarm):
                fill_ps = psF.tile([128, CHUNK], F32, tag="fill")
                nc.tensor.matmul(
                    fill_ps, wl_sb[:, 0:128], wt_sb[:, 0:CHUNK],
                    start=True, stop=True,
                )
            for ck in range(n_chunks):
                if ck + PREFETCH + 1 < n_chunks:
                    load(ck + PREFETCH + 1)
                cs = chunk_sizes[ck]
                h = 2 * cs
                mt, ft = m_tiles.pop(ck), f_tiles.pop(ck)
                ot = opool.tile([128, 4 * CHUNK], BF16, tag="o")
                runs = chunk_runs[ck]
                # route the final chunks' stores to the by-then-idle load
                # queues so the tail drains in parallel
                if last_hwdge and ck >= n_chunks - last_hwdge:
                    st1, st2 = nc.sync, nc.scalar
                else:
                    st1 = st2 = nc.gpsimd
                for c in range(4):
                    ct = 0 if c == 0 else 1
                    y_ps = psY.tile([128, CHUNK], F32, tag="y")
                    nc.tensor.matmul(
                        y_ps[:, 0:cs],
                        wl_sb[:, ct * 128:(ct + 1) * 128],
                        mt[:, c * cs:(c + 1) * cs],
                        start=True,
                        stop=False,
                    )
                    for ri, (off, size, g) in enumerate(runs):
                        nc.tensor.matmul(
                            y_ps[:, off:off + size],
                            wt_sb[:, (ct * NA + g) * 128:(ct * NA + g + 1) * 128],
                            ft[:, c * cs + off:c * cs + off + size],
                            start=False,
                            stop=(ri == len(runs) - 1),
                        )
                    if copy_split and c % 2 == 0:
                        nc.scalar.copy(ot[:, c * cs:(c + 1) * cs], y_ps[:, 0:cs])
                    else:
                        nc.vector.tensor_copy(ot[:, c * cs:(c + 1) * cs], y_ps[:, 0:cs])
                    if c == 1:
                        st1.dma_start(out=out[ck][:, 0:h], in_=ot[:, 0:h])
                # fillers: dep-free matmuls on a scratch bank that run in the
                # tail-of-chunk PE idle, keeping HAM activity windows above
                # the K=4/8 demotion threshold
                for _ in range(fillers):
                    fill_ps = psF.tile([128, fillw], F32, tag="fill")
                    nc.tensor.matmul(
                        fill_ps, wl_sb[:, 0:128], wt_sb[:, 0:fillw],
                        start=True, stop=True,
                    )
                if qlast and ck == n_chunks - 1:
                    # split the very last half-store across two queues
                    q = h // 2
                    nc.scalar.dma_start(out=out[ck][:, h:h + q], in_=ot[:, h:h + q])
                    nc.gpsimd.dma_start(out=out[ck][:, h + q:2 * h], in_=ot[:, h + q:2 * h])
                else:
                    st2.dma_start(out=out[ck][:, h:2 * h], in_=ot[:, h:2 * h])

    nc.finalize()
    return nc


def build_program_dense(n_tiles, use_f32r=True):
    """Fallback for non-one-hot attrs: dense sum over the NA attr channels.

    Extra input: att [S, NA]. f32 node-major layout with on-chip PE
    transposes (slow but correct for arbitrary attrs).
    """
    assert n_tiles % 4 == 0
    S = n_tiles * 128
    nc = bacc.Bacc("TRN2")
    m = nc.dram_tensor("m", [S, DIM], F32, kind="ExternalInput")
    f = nc.dram_tensor("f", [S, DIM], F32, kind="ExternalInput")
    att = nc.dram_tensor("att", [S, NA], F32, kind="ExternalInput")
    wl = nc.dram_tensor("wl", [MUL, 2 * MUL], F32, kind="ExternalInput")
    wt = nc.dram_tensor("wt", [MUL, 2 * NA * MUL], F32, kind="ExternalInput")
    ident = nc.dram_tensor("ident", [MUL, MUL], F32, kind="ExternalInput")
    out = nc.dram_tensor("out", [S, DIM], F32, kind="ExternalOutput")

    mm_dt = F32R if use_f32r else F32

    with TileContext(nc) as tc:
        with (
            tc.tile_pool(name="const", bufs=1) as cpool,
            tc.tile_pool(name="io", bufs=6) as iopool,
            tc.tile_pool(name="gsb", bufs=44) as gpool,
            tc.tile_pool(name="tmp", bufs=3) as tmpool,
            tc.tile_pool(name="tsb", bufs=12) as tpool,
            tc.tile_pool(name="ysb", bufs=6) as ypool,
            tc.tile_pool(name="psA", bufs=2, space="PSUM") as psA,
            tc.tile_pool(name="psB", bufs=2, space="PSUM") as psB,
            tc.tile_pool(name="psY", bufs=2, space="PSUM") as psY,
            tc.tile_pool(name="psO", bufs=2, space="PSUM") as psO,
        ):
            wl_sb = cpool.tile([MUL, 2 * MUL], F32, tag="wl")
            nc.sync.dma_start(out=wl_sb, in_=wl[:])
            wt_sb = cpool.tile([MUL, 2 * NA * MUL], F32, tag="wt")
            nc.sync.dma_start(out=wt_sb, in_=wt[:])
            id_sb = cpool.tile([MUL, MUL], F32, tag="ident")
            nc.sync.dma_start(out=id_sb, in_=ident[:])
            if use_f32r:
                wlr = cpool.tile([MUL, 2 * MUL], mm_dt, tag="wlr")
                nc.vector.tensor_copy(wlr, wl_sb)
                wtr = cpool.tile([MUL, 2 * NA * MUL], mm_dt, tag="wtr")
                nc.vector.tensor_copy(wtr, wt_sb)
                wl_sb, wt_sb = wlr, wtr

            for ck in range(n_tiles // 4):
                t0 = ck * 4
                m_tiles, f_tiles, a_tiles = [], [], []
                for nb in range(4):
                    mt = iopool.tile([128, DIM], F32, tag="m_sb")
                    nc.sync.dma_start(out=mt, in_=m[(t0 + nb) * 128:(t0 + nb + 1) * 128])
                    m_tiles.append(mt)
                    ft = iopool.tile([128, DIM], F32, tag="f_sb")
                    nc.sync.dma_start(out=ft, in_=f[(t0 + nb) * 128:(t0 + nb + 1) * 128])
                    f_tiles.append(ft)
                    at = iopool.tile([128, NA], F32, tag="a_sb")
                    nc.sync.dma_start(out=at, in_=att[(t0 + nb) * 128:(t0 + nb + 1) * 128])
                    a_tiles.append(at)

                # pre-scale: g[a][nb] = f[nb] * att[:, a]
                g_tiles = []
                for a in range(NA):
                    row = []
                    for nb in range(4):
                        gt = gpool.tile([128, DIM], F32, tag="g_sb")
                        nc.vector.tensor_scalar_mul(gt, f_tiles[nb], a_tiles[nb][:, a:a + 1])
                        row.append(gt)
                    g_tiles.append(row)

                y_sbs = []
                for c in range(4):
                    ct = 0 if c == 0 else 1
                    tm_ps = psA.tile([128, 512], F32, tag="tm_ps")
                    for nb in range(4):
                        nc.tensor.matmul(
                            tm_ps[:, nb * 128:(nb + 1) * 128],
                            m_tiles[nb][:, c * 128:(c + 1) * 128],
                            id_sb, is_transpose=True,
                        )
                    tm_sb = tmpool.tile([128, 512], mm_dt, tag="tm_sb")
                    nc.scalar.copy(tm_sb, tm_ps)

                    tg_sbs = []
                    for a in range(NA):
                        tg_ps = psB.tile([128, 512], F32, tag="tg_ps")
                        for nb in range(4):
                            nc.tensor.matmul(
                                tg_ps[:, nb * 128:(nb + 1) * 128],
                                g_tiles[a][nb][:, c * 128:(c + 1) * 128],
                                id_sb, is_transpose=True,
                            )
                        tg_sb = tpool.tile([128, 512], mm_dt, tag="tg_sb")
                        if a % 2 == 0:
                            nc.scalar.copy(tg_sb, tg_ps)
                        else:
                            nc.vector.tensor_copy(tg_sb, tg_ps)
                        tg_sbs.append(tg_sb)
                    y_ps = psY.tile([128, 512], F32, tag="y_ps")
                    nc.tensor.matmul(
                        y_ps,
                        wl_sb[:, ct * 128:(ct + 1) * 128],
                        tm_sb,
                        start=True, stop=False,
                    )
                    for a in range(NA):
                        nc.tensor.matmul(
                            y_ps,
                            wt_sb[:, (ct * NA + a) * 128:(ct * NA + a + 1) * 128],
                            tg_sbs[a],
                            start=False, stop=(a == NA - 1),
                        )
                    y_sb = ypool.tile([128, 512], F32, tag="y_sb")
                    if c % 2 == 0:
                        nc.scalar.copy(y_sb, y_ps)
                    else:
                        nc.vector.tensor_copy(y_sb, y_ps)
                    y_sbs.append(y_sb)

                for nb in range(4):
                    o_ps = psO.tile([128, 512], F32, tag="o_ps")
                    for c in range(4):
                        nc.tensor.matmul(
                            o_ps[:, c * 128:(c + 1) * 128],
                            y_sbs[c][:, nb * 128:(nb + 1) * 128],
                            id_sb, is_transpose=True,
                        )
                    o_sb = iopool.tile([128, DIM], F32, tag="o_sb")
                    if nb % 2 == 0:
                        nc.scalar.copy(o_sb, o_ps)
                    else:
                        nc.vector.tensor_copy(o_sb, o_ps)
                    nc.sync.dma_start(
                        out=out[(t0 + nb) * 128:(t0 + nb + 1) * 128], in_=o_sb
                    )

    nc.finalize()
    return nc


def pack_weights(Wl0, Wl1, Wt0, Wt1, np_dt):
    wl = np.concatenate([Wl0 * LIN_SCALE, Wl1 * LIN_SCALE], axis=1).astype(np_dt)
    blocks = [Wt0[:, a, :] * TP_SCALE for a in range(NA)] + [
        Wt1[:, a, :] * TP_SCALE for a in range(NA)
    ]
    wt = np.concatenate(blocks, axis=1).astype(np_dt)
    return np.ascontiguousarray(wl), np.ascontiguousarray(wt)


def plan_grouped(node_attrs):
    """One-hot grouping/sharding plan, or None if attrs are not one-hot.

    Each attr group is split evenly over the cores and padded (per core)
    to the max per-core share so the same program runs on every core;
    group boundaries may fall anywhere inside a chunk (matmul runs use
    arbitrary free-dim offsets).
    """
    N = node_attrs.shape[0]
    z = np.argmax(node_attrs, axis=1)
    onehot = np.zeros_like(node_attrs)
    onehot[np.arange(N), z] = 1.0
    if not np.array_equal(node_attrs, onehot):
        return None

    order = np.argsort(z, kind="stable")
    counts = np.bincount(z, minlength=NA)
    k = -(-counts // N_CORES)  # ceil: per-core padded group size
    S0 = int(k.sum())
    nfull = S0 // CHUNK
    rem = S0 - nfull * CHUNK
    chunk_sizes = [CHUNK] * nfull
    if rem:
        chunk_sizes.append(-(-rem // 32) * 32)  # ragged tail chunk
    S = int(sum(chunk_sizes))
    goff = np.concatenate([[0], np.cumsum(k)])

    # group intervals covering [0, S): extend the last non-empty group
    intervals = [(int(goff[a]), int(goff[a + 1]), a) for a in range(NA) if k[a] > 0]
    lo, hi, g = intervals[-1]
    intervals[-1] = (lo, S, g)

    chunk_runs = []
    c0 = 0
    for cs in chunk_sizes:
        c1 = c0 + cs
        runs = []
        for lo, hi, g in intervals:
            s, e = max(lo, c0), min(hi, c1)
            if s < e:
                runs.append((s - c0, e - s, g))
        chunk_runs.append(tuple(runs))
        c0 = c1

    per_core_idx = [[] for _ in range(N_CORES)]
    per_core_pos = [[] for _ in range(N_CORES)]
    pos = 0
    for a in range(NA):
        ga = order[pos:pos + counts[a]]
        pos += counts[a]
        q, r = divmod(len(ga), N_CORES)
        off = 0
        for cidx in range(N_CORES):
            take = q + (1 if cidx < r else 0)
            per_core_idx[cidx].append(ga[off:off + take])
            per_core_pos[cidx].append(goff[a] + np.arange(take))
            off += take

    plans = []
    for cidx in range(N_CORES):
        idx = np.concatenate(per_core_idx[cidx])
        posn = np.concatenate(per_core_pos[cidx]).astype(np.int64)
        plans.append((idx, posn))
    return dict(S=S, chunk_sizes=tuple(chunk_sizes),
                chunk_runs=tuple(chunk_runs), plans=plans)


def pack_tiled(rows_bf16, posn, chunk_sizes, np_dt=NP_BF16):
    """[n_rows, 512] bf16 (plane-major cols) -> [n_chunks, 128, 2048] tiled.

    Full chunks hold [128, 4, 512]; a ragged last chunk is packed at
    [128, 4, cs] within the leading 4*cs cols of its block.
    """
    S = int(sum(chunk_sizes))
    n_chunks = len(chunk_sizes)
    pad = np.zeros((S, DIM), dtype=np_dt)
    pad[posn] = rows_bf16
    buf = np.zeros((n_chunks, 128, 4 * CHUNK), dtype=np_dt)
    nfull = sum(1 for cs in chunk_sizes if cs == CHUNK)
    if nfull:
        t = pad[:nfull * CHUNK].reshape(nfull, CHUNK, 4, 128).transpose(0, 3, 2, 1)
        buf[:nfull] = np.ascontiguousarray(t).reshape(nfull, 128, 4 * CHUNK)
    if nfull < n_chunks:
        cs = chunk_sizes[-1]
        t = pad[nfull * CHUNK:].reshape(cs, 4, 128).transpose(2, 1, 0)
        buf[-1][:, :4 * cs] = np.ascontiguousarray(t).reshape(128, 4 * cs)
    return buf


def unpack_tiled(tiled, posn, chunk_sizes):
    """[n_chunks, 128, 2048] bf16 -> rows [len(posn), 512] bf16 plane-major."""
    S = int(sum(chunk_sizes))
    n_chunks = len(chunk_sizes)
    nfull = sum(1 for cs in chunk_sizes if cs == CHUNK)
    y = np.empty((S, DIM), dtype=NP_BF16)
    if nfull:
        t = tiled[:nfull].reshape(nfull, 128, 4, CHUNK).transpose(0, 3, 2, 1)
        y[:nfull * CHUNK] = np.ascontiguousarray(t).reshape(nfull * CHUNK, DIM)
    if nfull < n_chunks:
        cs = chunk_sizes[-1]
        t = tiled[-1][:, :4 * cs].reshape(128, 4, cs).transpose(2, 1, 0)
        y[nfull * CHUNK:] = np.ascontiguousarray(t).reshape(cs, DIM)
    return y[posn]


_CACHE = {}


def kernel(m_i, node_feats, node_attrs, Wl0, Wl1, Wt0, Wt1):
    global LAST_RESULTS
    import os
    trace = bool(os.environ.get("KERNEL_TRACE"))
    m_i = np.ascontiguousarray(m_i, dtype=np.float32)
    node_feats = np.ascontiguousarray(node_feats, dtype=np.float32)
    node_attrs = np.ascontiguousarray(node_attrs, dtype=np.float32)
    N = m_i.shape[0]

    plan = plan_grouped(node_attrs)
    if plan is not None:
        wl, wt = pack_weights(Wl0, Wl1, Wt0, Wt1, NP_BF16)
        key = ("grouped-fp8f", plan["chunk_sizes"], plan["chunk_runs"])
        if key not in _CACHE:
            _CACHE.clear()
            _CACHE[key] = build_program(plan["chunk_sizes"], plan["chunk_runs"])
        nc = _CACHE[key]
        chunk_sizes = plan["chunk_sizes"]
        mp = m_i.astype(NP_BF16)[:, COL_PERM]
        fp = node_feats.astype(NP_FP8)[:, COL_PERM]
        in_maps = []
        for cidx in range(N_CORES):
            idx, posn = plan["plans"][cidx]
            in_maps.append(dict(
                m=pack_tiled(mp[idx], posn, chunk_sizes),
                f=pack_tiled(fp[idx], posn, chunk_sizes, NP_FP8),
                wl=wl, wt=wt,
            ))
        res = run_bass_kernel_spmd(
            nc, in_maps, core_ids=list(range(N_CORES)), trace=trace
        )
        LAST_RESULTS = res
        out = np.empty((N, DIM), dtype=np.float32)
        for cidx in range(N_CORES):
            idx, posn = plan["plans"][cidx]
            rows = unpack_tiled(res.results[cidx]["out"], posn, chunk_sizes)
            out[idx] = rows.astype(np.float32)[:, COL_PERM_INV]
        return out

    # dense fallback (arbitrary attrs)
    wl, wt = pack_weights(Wl0, Wl1, Wt0, Wt1, np.float32)
    ident = np.eye(128, dtype=np.float32)
    mp = np.ascontiguousarray(m_i[:, COL_PERM])
    fp = np.ascontiguousarray(node_feats[:, COL_PERM])
    per_core = max(512, int(np.ceil(N / N_CORES / 512.0)) * 512)
    S = per_core
    key = ("dense", S)
    if key not in _CACHE:
        _CACHE.clear()
        _CACHE[key] = build_program_dense(S // 128)
    nc = _CACHE[key]
    in_maps = []
    bounds = []
    for cidx in range(N_CORES):
        lo = min(cidx * per_core, N)
        hi = min(lo + per_core, N)
        mpad = np.zeros((S, DIM), dtype=np.float32)
        fpad = np.zeros((S, DIM), dtype=np.float32)
        apad = np.zeros((S, NA), dtype=np.float32)
        mpad[:hi - lo] = mp[lo:hi]
        fpad[:hi - lo] = fp[lo:hi]
        apad[:hi - lo] = node_attrs[lo:hi]
        in_maps.append(dict(m=mpad, f=fpad, att=apad, wl=wl, wt=wt, ident=ident))
        bounds.append((lo, hi))
    res = run_bass_kernel_spmd(
        nc, in_maps, core_ids=list(range(N_CORES)), trace=trace
    )
    LAST_RESULTS = res
    out = np.empty((N, DIM), dtype=np.float32)
    for cidx, (lo, hi) in enumerate(bounds):
        out[lo:hi] = res.results[cidx]["out"][:hi - lo]
    return np.ascontiguousarray(out[:, COL_PERM_INV])


# revision 27
# speedup vs baseline: 1.1904x; 1.0177x over previous
"""Trainium2 (8-core) kernel for nn_NodeUpdateBlock: equivariant Linear +
FullyConnectedTensorProduct with 10 scalar (0e) one-hot attributes.

Self-contained: takes FULL inputs (as produced by the problem's
setup_inputs), distributes across the 8 NeuronCores internally, and
returns the FULL [N, 512] float32 output.

Strategy
--------
out_c = m_c @ (Wl_c * ls) + sum_a (att_a * f_c) @ (Wt_c[:,a,:] * ts)
per irrep-component plane c (1 scalar plane + 3 vector planes of 128
channels).  node_attrs rows are one-hot, so on the host we compute
z = argmax(attrs), sort nodes by z, and split every attr-group evenly
over the 8 cores; the tensor product then needs exactly ONE matmul per
(plane, group-run) with a compile-time weight slice.

The kernel is memory-bound, so the data plane is narrow: the host
downcasts m (and the weights) to bf16 and f to fp8-e4m3 (the TP term
carries only ~9% of the output energy, so fp8 noise on f costs ~0.8%
rel err total), transposes to channel-major,
and pre-tiles into the exact SBUF layout [chunk, 128 part, plane, 512
nodes] so every DMA packet is a contiguous 4 KiB run on both sides and
the PE needs no on-chip transposes at all.  Per 512-node chunk and
plane, one PSUM accumulation group computes
Y^T = WL^T m^T + WT_g^T f^T, and the result is copied to SBUF as bf16
and streamed out.  The host un-tiles and upcasts to f32 (bf16 rel-err
~3e-3, well inside the 2e-2 gate).  A dense (non-one-hot) f32 fallback
sums over all 10 attribute channels and stays correct for arbitrary
node_attrs.
"""

import math

import numpy as np
import ml_dtypes

import concourse.bacc as bacc
import concourse.mybir as mybir
from concourse.tile import TileContext
from concourse.bass_utils import run_bass_kernel_spmd

MUL = 128
NA = 10
DIM = 512
N_CORES = 8
CHUNK = 512  # nodes per compute chunk
LIN_SCALE = 1.0 / math.sqrt(MUL)
TP_SCALE = 1.0 / math.sqrt(MUL * NA)
F32 = mybir.dt.float32
F32R = mybir.dt.float32r
BF16 = mybir.dt.bfloat16
FP8 = mybir.dt.float8e4
NP_BF16 = ml_dtypes.bfloat16
NP_FP8 = ml_dtypes.float8_e4m3

LAST_RESULTS = None  # BassKernelResults of the most recent run (for testing)


def _col_perm():
    perm = list(range(MUL))
    for x in range(3):
        perm += [MUL + 3 * i + x for i in range(MUL)]
    return np.array(perm, dtype=np.int64)


COL_PERM = _col_perm()
COL_PERM_INV = np.argsort(COL_PERM)


def build_program(chunk_sizes, chunk_runs, edge_opt=False, fillers=2, prefetch=4, copy_split=False, prewarm=0, last_hwdge=2, fillw=256, psy=7, early_gpl=0, qlast=False):
    """Bass program for one core (same program runs on all cores).

    Inputs:  m/f [n_chunks, 128, 2048] bf16 (pre-tiled channel-major; the
             last chunk may be ragged, packed at 4*cs cols within its block),
             wl [128, 256] bf16, wt [128, 2560] bf16
    Output:  out [n_chunks, 128, 2048] bf16 (same tiling)
    """
    n_chunks = len(chunk_sizes)
    nc = bacc.Bacc("TRN2")
    m = nc.dram_tensor("m", [n_chunks, 128, 4 * CHUNK], BF16, kind="ExternalInput")
    f = nc.dram_tensor("f", [n_chunks, 128, 4 * CHUNK], FP8, kind="ExternalInput")
    wl = nc.dram_tensor("wl", [MUL, 2 * MUL], BF16, kind="ExternalInput")
    wt = nc.dram_tensor("wt", [MUL, 2 * NA * MUL], BF16, kind="ExternalInput")
    out = nc.dram_tensor("out", [n_chunks, 128, 4 * CHUNK], BF16, kind="ExternalOutput")

    with TileContext(nc) as tc:
        with (
            tc.tile_pool(name="const", bufs=1) as cpool,
            tc.tile_pool(name="mio", bufs=7) as mpool,
            tc.tile_pool(name="fio", bufs=7) as fpool,
            tc.tile_pool(name="oio", bufs=6) as opool,
            tc.tile_pool(name="psY", bufs=psy, space="PSUM") as psY,
            tc.tile_pool(name="psF", bufs=1, space="PSUM") as psF,
        ):
            # weights ride the (otherwise idle at t=0) gpsimd queue so the
            # sync/scalar queues start streaming chunk data immediately
            wl_sb = cpool.tile([MUL, 2 * MUL], BF16, tag="wl")
            nc.gpsimd.dma_start(out=wl_sb, in_=wl[:])
            wt_sb = cpool.tile([MUL, 2 * NA * MUL], BF16, tag="wt")
            nc.gpsimd.dma_start(out=wt_sb, in_=wt[:])

            # loads split half/half across the two HWDGE queues; stores on
            # gpsimd.  Loads are issued PREFETCH chunks ahead so their
            # issue slots never queue behind PSUM-copy waits on the same
            # engine.
            PREFETCH = prefetch
            m_tiles, f_tiles = {}, {}

            def load(ck):
                cs = chunk_sizes[ck]
                h = 2 * cs
                mt = mpool.tile([128, 4 * CHUNK], BF16, tag="m")
                ft = fpool.tile([128, 4 * CHUNK], FP8, tag="f")
                # the store queue (gpsimd) is idle during the ramp: let it
                # carry the second halves of the first two chunks
                eng2 = nc.gpsimd if ck < early_gpl else nc.scalar
                nc.sync.dma_start(out=mt[:, 0:h], in_=m[ck][:, 0:h])
                eng2.dma_start(out=mt[:, h:2 * h], in_=m[ck][:, h:2 * h])
                nc.sync.dma_start(out=ft[:, 0:h], in_=f[ck][:, 0:h])
                eng2.dma_start(out=ft[:, h:2 * h], in_=f[ck][:, h:2 * h])
                m_tiles[ck], f_tiles[ck] = mt, ft

            for ck in range(min(PREFETCH + 1, n_chunks)):
                load(ck)
            # pre-warm HAM before the first chunk's matmuls arrive
            for _ in range(prewarm):
                fill_ps = psF.tile([128, CHUNK], F32, tag="fill")
                nc.tensor.matmul(
                    fill_ps, wl_sb[:, 0:128], wt_sb[:, 0:CHUNK],
                    start=True, stop=True,
                )
            for ck in range(n_chunks):
                if ck + PREFETCH + 1 < n_chunks:
                    load(ck + PREFETCH + 1)
                cs = chunk_sizes[ck]
                h = 2 * cs
                mt, ft = m_tiles.pop(ck), f_tiles.pop(ck)
                ot = opool.tile([128, 4 * CHUNK], BF16, tag="o")
                runs = chunk_runs[ck]
                # route the final chunks' stores to the by-then-idle load
                # queues so the tail drains in parallel
                if last_hwdge and ck >= n_chunks - last_hwdge:
                    st1, st2 = nc.sync, nc.scalar
                else:
                    st1 = st2 = nc.gpsimd
                for c in range(4):
                    ct = 0 if c == 0 else 1
                    y_ps = psY.tile([128, CHUNK], F32, tag="y")
                    nc.tensor.matmul(
                        y_ps[:, 0:cs],
                        wl_sb[:, ct * 128:(ct + 1) * 128],
                        mt[:, c * cs:(c + 1) * cs],
                        start=True,
                        stop=False,
                    )
                    for ri, (off, size, g) in enumerate(runs):
                        nc.tensor.matmul(
                            y_ps[:, off:off + size],
                            wt_sb[:, (ct * NA + g) * 128:(ct * NA + g + 1) * 128],
                            ft[:, c * cs + off:c * cs + off + size],
                            start=False,
                            stop=(ri == len(runs) - 1),
                        )
                    if copy_split and c % 2 == 0:
                        nc.scalar.copy(ot[:, c * cs:(c + 1) * cs], y_ps[:, 0:cs])
                    else:
                        nc.vector.tensor_copy(ot[:, c * cs:(c + 1) * cs], y_ps[:, 0:cs])
                    if c == 1:
                        st1.dma_start(out=out[ck][:, 0:h], in_=ot[:, 0:h])
                # fillers: dep-free matmuls on a scratch bank that run in the
                # tail-of-chunk PE idle, keeping HAM activity windows above
                # the K=4/8 demotion threshold
                for _ in range(fillers):
                    fill_ps = psF.tile([128, fillw], F32, tag="fill")
                    nc.tensor.matmul(
                        fill_ps, wl_sb[:, 0:128], wt_sb[:, 0:fillw],
                        start=True, stop=True,
                    )
                if qlast and ck == n_chunks - 1:
                    # split the very last half-store across two queues
                    q = h // 2
                    nc.scalar.dma_start(out=out[ck][:, h:h + q], in_=ot[:, h:h + q])
                    nc.gpsimd.dma_start(out=out[ck][:, h + q:2 * h], in_=ot[:, h + q:2 * h])
                else:
                    st2.dma_start(out=out[ck][:, h:2 * h], in_=ot[:, h:2 * h])

    nc.finalize()
    return nc


def build_program_dense(n_tiles, use_f32r=True):
    """Fallback for non-one-hot attrs: dense sum over the NA attr channels.

    Extra input: att [S, NA]. f32 node-major layout with on-chip PE
    transposes (slow but correct for arbitrary attrs).
    """
    assert n_tiles % 4 == 0
    S = n_tiles * 128
    nc = bacc.Bacc("TRN2")
    m = nc.dram_tensor("m", [S, DIM], F32, kind="ExternalInput")
    f = nc.dram_tensor("f", [S, DIM], F32, kind="ExternalInput")
    att = nc.dram_tensor("att", [S, NA], F32, kind="ExternalInput")
    wl = nc.dram_tensor("wl", [MUL, 2 * MUL], F32, kind="ExternalInput")
    wt = nc.dram_tensor("wt", [MUL, 2 * NA * MUL], F32, kind="ExternalInput")
    ident = nc.dram_tensor("ident", [MUL, MUL], F32, kind="ExternalInput")
    out = nc.dram_tensor("out", [S, DIM], F32, kind="ExternalOutput")

    mm_dt = F32R if use_f32r else F32

    with TileContext(nc) as tc:
        with (
            tc.tile_pool(name="const", bufs=1) as cpool,
            tc.tile_pool(name="io", bufs=6) as iopool,
            tc.tile_pool(name="gsb", bufs=44) as gpool,
            tc.tile_pool(name="tmp", bufs=3) as tmpool,
            tc.tile_pool(name="tsb", bufs=12) as tpool,
            tc.tile_pool(name="ysb", bufs=6) as ypool,
            tc.tile_pool(name="psA", bufs=2, space="PSUM") as psA,
            tc.tile_pool(name="psB", bufs=2, space="PSUM") as psB,
            tc.tile_pool(name="psY", bufs=2, space="PSUM") as psY,
            tc.tile_pool(name="psO", bufs=2, space="PSUM") as psO,
        ):
            wl_sb = cpool.tile([MUL, 2 * MUL], F32, tag="wl")
            nc.sync.dma_start(out=wl_sb, in_=wl[:])
            wt_sb = cpool.tile([MUL, 2 * NA * MUL], F32, tag="wt")
            nc.sync.dma_start(out=wt_sb, in_=wt[:])
            id_sb = cpool.tile([MUL, MUL], F32, tag="ident")
            nc.sync.dma_start(out=id_sb, in_=ident[:])
            if use_f32r:
                wlr = cpool.tile([MUL, 2 * MUL], mm_dt, tag="wlr")
                nc.vector.tensor_copy(wlr, wl_sb)
                wtr = cpool.tile([MUL, 2 * NA * MUL], mm_dt, tag="wtr")
                nc.vector.tensor_copy(wtr, wt_sb)
                wl_sb, wt_sb = wlr, wtr

            for ck in range(n_tiles // 4):
                t0 = ck * 4
                m_tiles, f_tiles, a_tiles = [], [], []
                for nb in range(4):
                    mt = iopool.tile([128, DIM], F32, tag="m_sb")
                    nc.sync.dma_start(out=mt, in_=m[(t0 + nb) * 128:(t0 + nb + 1) * 128])
                    m_tiles.append(mt)
                    ft = iopool.tile([128, DIM], F32, tag="f_sb")
                    nc.sync.dma_start(out=ft, in_=f[(t0 + nb) * 128:(t0 + nb + 1) * 128])
                    f_tiles.append(ft)
                    at = iopool.tile([128, NA], F32, tag="a_sb")
                    nc.sync.dma_start(out=at, in_=att[(t0 + nb) * 128:(t0 + nb + 1) * 128])
                    a_tiles.append(at)

                # pre-scale: g[a][nb] = f[nb] * att[:, a]
                g_tiles = []
                for a in range(NA):
                    row = []
                    for nb in range(4):
                        gt = gpool.tile([128, DIM], F32, tag="g_sb")
                        nc.vector.tensor_scalar_mul(gt, f_tiles[nb], a_tiles[nb][:, a:a + 1])
                        row.append(gt)
                    g_tiles.append(row)

                y_sbs = []
                for c in range(4):
                    ct = 0 if c == 0 else 1
                    tm_ps = psA.tile([128, 512], F32, tag="tm_ps")
                    for nb in range(4):
                        nc.tensor.matmul(
                            tm_ps[:, nb * 128:(nb + 1) * 128],
                            m_tiles[nb][:, c * 128:(c + 1) * 128],
                            id_sb, is_transpose=True,
                        )
                    tm_sb = tmpool.tile([128, 512], mm_dt, tag="tm_sb")
                    nc.scalar.copy(tm_sb, tm_ps)

                    tg_sbs = []
                    for a in range(NA):
                        tg_ps = psB.tile([128, 512], F32, tag="tg_ps")
                        for nb in range(4):
                            nc.tensor.matmul(
                                tg_ps[:, nb * 128:(nb + 1) * 128],
                                g_tiles[a][nb][:, c * 128:(c + 1) * 128],
                                id_sb, is_transpose=True,
                            )
                        tg_sb = tpool.tile([128, 512], mm_dt, tag="tg_sb")
                        if a % 2 == 0:
                            nc.scalar.copy(tg_sb, tg_ps)
                        else:
                            nc.vector.tensor_copy(tg_sb, tg_ps)
                        tg_sbs.append(tg_sb)
                    y_ps = psY.tile([128, 512], F32, tag="y_ps")
                    nc.tensor.matmul(
                        y_ps,
                        wl_sb[:, ct * 128:(ct + 1) * 128],
                        tm_sb,
                        start=True, stop=False,
                    )
                    for a in range(NA):
                        nc.tensor.matmul(
                            y_ps,
                            wt_sb[:, (ct * NA + a) * 128:(ct * NA + a + 1) * 128],
                            tg_sbs[a],
                            start=False, stop=(a == NA - 1),
                        )
                    y_sb = ypool.tile([128, 512], F32, tag="y_sb")
                    if c % 2 == 0:
                        nc.scalar.copy(y_sb, y_ps)
                    else:
                        nc.vector.tensor_copy(y_sb, y_ps)
                    y_sbs.append(y_sb)

                for nb in range(4):
                    o_ps = psO.tile([128, 512], F32, tag="o_ps")
                    for c in range(4):
                        nc.tensor.matmul(
                            o_ps[:, c * 128:(c + 1) * 128],
                            y_sbs[c][:, nb * 128:(nb + 1) * 128],
                            id_sb, is_transpose=True,
                        )
                    o_sb = iopool.tile([128, DIM], F32, tag="o_sb")
                    if nb % 2 == 0:
                        nc.scalar.copy(o_sb, o_ps)
                    else:
                        nc.vector.tensor_copy(o_sb, o_ps)
                    nc.sync.dma_start(
                        out=out[(t0 + nb) * 128:(t0 + nb + 1) * 128], in_=o_sb
                    )

    nc.finalize()
    return nc


def pack_weights(Wl0, Wl1, Wt0, Wt1, np_dt):
    wl = np.concatenate([Wl0 * LIN_SCALE, Wl1 * LIN_SCALE], axis=1).astype(np_dt)
    blocks = [Wt0[:, a, :] * TP_SCALE for a in range(NA)] + [
        Wt1[:, a, :] * TP_SCALE for a in range(NA)
    ]
    wt = np.concatenate(blocks, axis=1).astype(np_dt)
    return np.ascontiguousarray(wl), np.ascontiguousarray(wt)


def plan_grouped(node_attrs):
    """One-hot grouping/sharding plan, or None if attrs are not one-hot.

    Each attr group is split evenly over the cores and padded (per core)
    to the max per-core share so the same program runs on every core;
    group boundaries may fall anywhere inside a chunk (matmul runs use
    arbitrary free-dim offsets).
    """
    N = node_attrs.shape[0]
    z = np.argmax(node_attrs, axis=1)
    onehot = np.zeros_like(node_attrs)
    onehot[np.arange(N), z] = 1.0
    if not np.array_equal(node_attrs, onehot):
        return None

    order = np.argsort(z, kind="stable")
    counts = np.bincount(z, minlength=NA)
    k = -(-counts // N_CORES)  # ceil: per-core padded group size
    S0 = int(k.sum())
    nfull = S0 // CHUNK
    rem = S0 - nfull * CHUNK
    chunk_sizes = [CHUNK] * nfull
    if rem:
        chunk_sizes.append(-(-rem // 32) * 32)  # ragged tail chunk
    S = int(sum(chunk_sizes))
    goff = np.concatenate([[0], np.cumsum(k)])

    # group intervals covering [0, S): extend the last non-empty group
    intervals = [(int(goff[a]), int(goff[a + 1]), a) for a in range(NA) if k[a] > 0]
    lo, hi, g = intervals[-1]
    intervals[-1] = (lo, S, g)

    chunk_runs = []
    c0 = 0
    for cs in chunk_sizes:
        c1 = c0 + cs
        runs = []
        for lo, hi, g in intervals:
            s, e = max(lo, c0), min(hi, c1)
            if s < e:
                runs.append((s - c0, e - s, g))
        chunk_runs.append(tuple(runs))
        c0 = c1

    per_core_idx = [[] for _ in range(N_CORES)]
    per_core_pos = [[] for _ in range(N_CORES)]
    pos = 0
    for a in range(NA):
        ga = order[pos:pos + counts[a]]
        pos += counts[a]
        q, r = divmod(len(ga), N_CORES)
        off = 0
        for cidx in range(N_CORES):
            take = q + (1 if cidx < r else 0)
            per_core_idx[cidx].append(ga[off:off + take])
            per_core_pos[cidx].append(goff[a] + np.arange(take))
            off += take

    plans = []
    for cidx in range(N_CORES):
        idx = np.concatenate(per_core_idx[cidx])
        posn = np.concatenate(per_core_pos[cidx]).astype(np.int64)
        plans.append((idx, posn))
    return dict(S=S, chunk_sizes=tuple(chunk_sizes),
                chunk_runs=tuple(chunk_runs), plans=plans)


def pack_tiled(rows_bf16, posn, chunk_sizes, np_dt=NP_BF16):
    """[n_rows, 512] bf16 (plane-major cols) -> [n_chunks, 128, 2048] tiled.

    Full chunks hold [128, 4, 512]; a ragged last chunk is packed at
    [128, 4, cs] within the leading 4*cs cols of its block.
    """
    S = int(sum(chunk_sizes))
    n_chunks = len(chunk_sizes)
    pad = np.zeros((S, DIM), dtype=np_dt)
    pad[posn] = rows_bf16
    buf = np.zeros((n_chunks, 128, 4 * CHUNK), dtype=np_dt)
    nfull = sum(1 for cs in chunk_sizes if cs == CHUNK)
    if nfull:
        t = pad[:nfull * CHUNK].reshape(nfull, CHUNK, 4, 128).transpose(0, 3, 2, 1)
        buf[:nfull] = np.ascontiguousarray(t).reshape(nfull, 128, 4 * CHUNK)
    if nfull < n_chunks:
        cs = chunk_sizes[-1]
        t = pad[nfull * CHUNK:].reshape(cs, 4, 128).transpose(2, 1, 0)
        buf[-1][:, :4 * cs] = np.ascontiguousarray(t).reshape(128, 4 * cs)
    return buf


def unpack_tiled(tiled, posn, chunk_sizes):
    """[n_chunks, 128, 2048] bf16 -> rows [len(posn), 512] bf16 plane-major."""
    S = int(sum(chunk_sizes))
    n_chunks = len(chunk_sizes)
    nfull = sum(1 for cs in chunk_sizes if cs == CHUNK)
    y = np.empty((S, DIM), dtype=NP_BF16)
    if nfull:
        t = tiled[:nfull].reshape(nfull, 128, 4, CHUNK).transpose(0, 3, 2, 1)
        y[:nfull * CHUNK] = np.ascontiguousarray(t).reshape(nfull * CHUNK, DIM)
    if nfull < n_chunks:
        cs = chunk_sizes[-1]
        t = tiled[-1][:, :4 * cs].reshape(128, 4, cs).transpose(2, 1, 0)
        y[nfull * CHUNK:] = np.ascontiguousarray(t).reshape(cs, DIM)
    return y[posn]


_CACHE = {}


def kernel(m_i, node_feats, node_attrs, Wl0, Wl1, Wt0, Wt1):
    global LAST_RESULTS
    import os
    trace = bool(os.environ.get("KERNEL_TRACE"))
    m_i = np.ascontiguousarray(m_i, dtype=np.float32)
    node_feats = np.ascontiguousarray(node_feats, dtype=np.float32)
    node_attrs = np.ascontiguousarray(node_attrs, dtype=np.float32)
    N = m_i.shape[0]

    plan = plan_grouped(node_attrs)
    if plan is not None:
        wl, wt = pack_weights(Wl0, Wl1, Wt0, Wt1, NP_BF16)
        key = ("grouped-fp8f", plan["chunk_sizes"], plan["chunk_runs"])
        if key not in _CACHE:
            _CACHE.clear()
            _CACHE[key] = build_program(plan["chunk_sizes"], plan["chunk_runs"])
        nc = _CACHE[key]
        chunk_sizes = plan["chunk_sizes"]
        mp = m_i.astype(NP_BF16)[:, COL_PERM]
        fp = node_feats.astype(NP_FP8)[:, COL_PERM]
        in_maps = []
        for cidx in range(N_CORES):
            idx, posn = plan["plans"][cidx]
            in_maps.append(dict(
                m=pack_tiled(mp[idx], posn, chunk_sizes),
                f=pack_tiled(fp[idx], posn, chunk_sizes, NP_FP8),
                wl=wl, wt=wt,
            ))
        res = run_bass_kernel_spmd(
            nc, in_maps, core_ids=list(range(N_CORES)), trace=trace
        )
        LAST_RESULTS = res
        out = np.empty((N, DIM), dtype=np.float32)
        for cidx in range(N_CORES):
            idx, posn = plan["plans"][cidx]
            rows = unpack_tiled(res.results[cidx]["out"], posn, chunk_sizes)
            out[idx] = rows.astype(np.float32)[:, COL_PERM_INV]
        return out

    # dense fallback (arbitrary attrs)
    wl, wt = pack_weights(Wl0, Wl1, Wt0, Wt1, np.float32)
    ident = np.eye(128, dtype=np.float32)
    mp = np.ascontiguousarray(m_i[:, COL_PERM])
    fp = np.ascontiguousarray(node_feats[:, COL_PERM])
    per_core = max(512, int(np.ceil(N / N_CORES / 512.0)) * 512)
    S = per_core
    key = ("dense", S)
    if key not in _CACHE:
        _CACHE.clear()
        _CACHE[key] = build_program_dense(S // 128)
    nc = _CACHE[key]
    in_maps = []
    bounds = []
    for cidx in range(N_CORES):
        lo = min(cidx * per_core, N)
        hi = min(lo + per_core, N)
        mpad = np.zeros((S, DIM), dtype=np.float32)
        fpad = np.zeros((S, DIM), dtype=np.float32)
        apad = np.zeros((S, NA), dtype=np.float32)
        mpad[:hi - lo] = mp[lo:hi]
        fpad[:hi - lo] = fp[lo:hi]
        apad[:hi - lo] = node_attrs[lo:hi]
        in_maps.append(dict(m=mpad, f=fpad, att=apad, wl=wl, wt=wt, ident=ident))
        bounds.append((lo, hi))
    res = run_bass_kernel_spmd(
        nc, in_maps, core_ids=list(range(N_CORES)), trace=trace
    )
    LAST_RESULTS = res
    out = np.empty((N, DIM), dtype=np.float32)
    for cidx, (lo, hi) in enumerate(bounds):
        out[lo:hi] = res.results[cidx]["out"][:hi - lo]
    return np.ascontiguousarray(out[:, COL_PERM_INV])
